# revision 2
# baseline (speedup 1.0000x reference)
"""Trainium2 Bass kernel for a ResNet Bottleneck block (inference).

Reference computation (NCHW, N=128, Cin=Cout=1024, width=256, H=W=14):
    out = relu(bn1(conv1x1(x, w1)))          # 1024 -> 256
    out = relu(bn2(conv3x3(out, w2, pad=1))) # 256 -> 256
    out = bn3(conv1x1(out, w3))              # 256 -> 1024
    y   = relu(out + x)

Strategy:
- Data-parallel: batch 128 sharded as 16 images per NeuronCore (8 cores),
  conv/BN params replicated. One NEFF, SPMD via run_bass_kernel_spmd.
- BN folded on host into per-channel weight scale + bias.
- All convs are matmuls on the TensorEngine with channels on the partition
  (contraction) dim. The 3x3 conv uses a zero-padded 16x16 per-image SBUF
  layout; each of the 9 taps is a shifted-window matmul accumulating in PSUM.
- Compute in bf16 (moving+stationary operands), fp32 PSUM accumulation,
  bf16 output (converted to fp32 on host).
- All input loads ride ONE HWDGE ring (sync) in exact consumption order:
  ring FIFO gives ordering for free at full HBM bandwidth, with no
  completion-chaining stalls. PE warm-up matmuls start right after the
  framework preamble so the HAM clock-gate lifts to 2.4 GHz before conv1.
- PSUM is managed as four [P, 1024] fp32 "pair" tiles (2 banks each): two
  accumulation chains per tile at column offsets 0 and 512. conv2/conv3
  evict two chains with ONE strided DVE/ACT op.
- Residual: 3 of 4 pairs per conv3 m-block go through DVE
  scalar_tensor_tensor (psum+bias)+x then ReLU on ACT/GpSimd; 1 pair stays
  on the PE as identity-weight matmuls so PE/DVE/ACT/GpSimd stay balanced.
"""

import sys

if "/opt/trn_rl_repo" not in sys.path:
    sys.path.insert(0, "/opt/trn_rl_repo")

import numpy as np
import ml_dtypes

import concourse.bass as bass
import concourse.bacc as bacc
import concourse.tile as tile
from concourse import mybir
from concourse.bass_utils import run_bass_kernel_spmd

EPS = 1e-5
NCORES = 8
NLOC = 16          # images per core
C_IN = 1024
WIDTH = 256
C_OUT = 1024
HW = 196           # 14*14
PADHW = 256        # 16*16 zero-padded image
P = 128
KB1 = C_IN // P    # 8 k-blocks for conv1 / residual channel blocks
KB2 = WIDTH // P   # 2 k-blocks for conv2/conv3 input
MB3 = C_OUT // P   # 8 m-blocks for conv3 output
NPAIRS = NLOC // 2  # 8 image pairs; N=392 per matmul
NF = 2 * HW        # 392
SLOT = 512         # fp32 columns per PSUM bank; chain s lives at s*SLOT

BF16 = mybir.dt.bfloat16
F32 = mybir.dt.float32
Relu = mybir.ActivationFunctionType.Relu

_cached = {}


def _build():
    """Build + compile the SPMD NEFF (one core's program). Cached."""
    if "nc" in _cached:
        return _cached["nc"]

    nc = bacc.Bacc("TRN2", target_bir_lowering=False, debug=False,
                   num_devices=NCORES)

    xt_d = nc.dram_tensor("xt", [2, KB1, P, NLOC * HW // 2], BF16,
                          kind="ExternalInput")
    # weights pre-arranged host-side as exact SBUF images (partition-major)
    w1_d = nc.dram_tensor("w1t", [P, KB1 * WIDTH], BF16, kind="ExternalInput")
    w2_d = nc.dram_tensor("w2t", [P, 9 * KB2 * WIDTH], BF16,
                          kind="ExternalInput")
    w3_d = nc.dram_tensor("w3t", [P, KB2 * C_OUT], BF16, kind="ExternalInput")
    b_d = nc.dram_tensor("biases", [P, 2 * KB2 + MB3], F32,
                         kind="ExternalInput")
    id_d = nc.dram_tensor("ident", [P, P], BF16, kind="ExternalInput")
    y_d = nc.dram_tensor("y", [MB3, P, NLOC * HW], BF16, kind="ExternalOutput")

    with tile.TileContext(nc) as tc:
        _emit(tc, nc, xt_d, w1_d, w2_d, w3_d, b_d, id_d, y_d)

    nc.compile()
    _cached["nc"] = nc
    return nc


def _emit(tc, nc, xt_d, w1_d, w2_d, w3_d, b_d, id_d, y_d):
    import contextlib
    from concourse.tile import add_dep_helper

    Alu = mybir.AluOpType

    with contextlib.ExitStack() as ctx:
        const = ctx.enter_context(tc.tile_pool(name="const", bufs=1))
        xpool = ctx.enter_context(tc.tile_pool(name="xpool", bufs=1))
        opool = ctx.enter_context(tc.tile_pool(name="opool", bufs=1))
        psp = ctx.enter_context(tc.tile_pool(name="psp", bufs=4, space="PSUM"))
        evp = ctx.enter_context(tc.tile_pool(name="evp", bufs=2))

        # ---- PE warm-up ---------------------------------------------------
        # The HAM clock gate needs ~3.4us of sustained PE activity to lift
        # the PE from 1.2 to 2.4 GHz. Start matmuls the moment the framework
        # preamble ends, on a DVE-memset scratch tile, alternating PSUM banks
        # so they pipeline at the issue rate.
        scratch = const.tile([P, SLOT], BF16, name="scratch", tag="scratch")
        nc.vector.memset(scratch[:], 0.0)
        warm_ps = psp.tile([P, 2 * SLOT], F32, name="warm_ps", tag="ps")
        for i in range(6):
            s = (i % 2) * SLOT
            nc.tensor.matmul(warm_ps[:, s:s + SLOT], scratch[:, 0:P],
                             scratch[:], start=True, stop=True)

        # ---- Input loads: one HWDGE ring, consumption order ---------------
        # HWDGE processes a ring FIFO, so emitting the loads in consumption
        # order gives prioritized, back-to-back transfers at full HBM
        # bandwidth with no semaphore waits. sync=False deps pin the issue
        # order without making later DMAs wait for earlier completions.
        ring_last = [None]

        def ring(eng, dst, src):
            i = eng.dma_start(dst, src)
            if ring_last[0] is not None:
                add_dep_helper(i.ins, ring_last[0], sync=False,
                               reason="dma ring order")
            ring_last[0] = i.ins
            return i

        xsb = xpool.tile([P, KB1 * NLOC * HW], BF16, name="xsb", tag="xsb")
        x_tiles = [xsb[:, k * NLOC * HW:(k + 1) * NLOC * HW]
                   for k in range(KB1)]
        xv = xsb[:].rearrange("p (k h c) -> p k h c", k=KB1, h=2)

        w1sb = const.tile([P, KB1 * WIDTH], BF16, name="w1sb", tag="w1sb")
        w1_t = [w1sb[:, k * WIDTH:(k + 1) * WIDTH] for k in range(KB1)]
        w2sb = const.tile([P, 9 * KB2 * WIDTH], BF16, name="w2sb", tag="w2sb")
        w2_t = [[w2sb[:, (tap * KB2 + k) * WIDTH:(tap * KB2 + k + 1) * WIDTH]
                 for k in range(KB2)] for tap in range(9)]
        w3sb = const.tile([P, KB2 * C_OUT], BF16, name="w3sb", tag="w3sb")
        w3_t = [w3sb[:, k * C_OUT:(k + 1) * C_OUT] for k in range(KB2)]

        def xload(half, k0, k1):
            ring(nc.sync, xv[:, k0:k1, half, :],
                 xt_d.ap()[half][k0:k1].rearrange("k p c -> p k c"))

        # w1 k0-1 first (needed by conv1's first matmuls), then x quad A
        # interleaved with the rest of w1, then quad B, then w2 in tap-major
        # thirds (conv2 contracts tap-outer), then w3.
        ring(nc.sync, w1sb[:, 0:2 * WIDTH], w1_d.ap()[:, 0:2 * WIDTH])
        xload(0, 0, 1)
        xload(0, 1, 2)
        ring(nc.sync, w1sb[:, 2 * WIDTH:], w1_d.ap()[:, 2 * WIDTH:])
        xload(0, 2, 4)
        xload(0, 4, 6)
        xload(0, 6, 8)
        xload(1, 0, 2)
        xload(1, 2, 4)
        xload(1, 4, 6)
        xload(1, 6, 8)
        W2C = 3 * KB2 * WIDTH
        for c in range(3):
            ring(nc.sync, w2sb[:, c * W2C:(c + 1) * W2C],
                 w2_d.ap()[:, c * W2C:(c + 1) * W2C])
        ring(nc.sync, w3sb[:], w3_d.ap())

        # small constants ride the other HWDGE ring (scalar), concurrently
        ball = const.tile([P, 2 * KB2 + MB3], F32, name="ball", tag="ball")
        nc.scalar.dma_start(ball[:], b_d.ap())
        b1_t = ball[:, 0:KB2]
        b2_t = ball[:, KB2:2 * KB2]
        b3_t = ball[:, 2 * KB2:]
        id_t = const.tile([P, P], BF16, name="id_t", tag="id_t")
        nc.scalar.dma_start(id_t[:], id_d.ap())

        # Zero-padded conv1 output: per image a 16x16 field, payload at
        # rows/cols 1..14. Layout [P, NLOC*256].
        out1 = []
        for m in range(KB2):
            t = opool.tile([P, NLOC * PADHW], BF16, name=f"out1_{m}",
                           tag=f"out1_{m}")
            nc.vector.memset(t[:], 0.0)
            out1.append(t)

        out2 = []
        for m in range(KB2):
            t = opool.tile([P, NLOC * HW], BF16, name=f"out2_{m}",
                           tag=f"out2_{m}")
            out2.append(t)

        def pad_view(k, np_):
            return (out1[k][:, np_ * 2 * PADHW:(np_ + 1) * 2 * PADHW]
                    .rearrange("p (i r c) -> p i r c", i=2, r=16, c=16))

        def pair_tiles(n, tag):
            return [psp.tile([P, 2 * SLOT], F32, name=f"{tag}_{j}", tag="ps")
                    for j in range(n)]

        def chain(t, s):
            return t[:, s * SLOT:s * SLOT + NF]

        # ---- conv1 (1x1, 1024->256) + bias + relu -> padded out1 ---------
        # Per np-quad: 8 chains in 4 pair tiles (pairs j x m), k outer so
        # tiles fill as x k-blocks land. Evictions are per-chain (padded
        # 4D dst), split DVE/ACT so each tile drains in one op-latency.
        for half in range(2):
            grp = {}
            for j in range(2):
                for m in range(KB2):
                    grp[(j, m)] = psp.tile([P, 2 * SLOT], F32,
                                           name=f"ps1_{j}_{m}", tag="ps")
            for k in range(KB1):
                for j in range(2):
                    for m in range(KB2):
                        for s in range(2):
                            np_ = half * 4 + 2 * j + s
                            nc.tensor.matmul(
                                chain(grp[(j, m)], s),
                                w1_t[k][:, m * P:(m + 1) * P],
                                x_tiles[k][:, np_ * NF:(np_ + 1) * NF],
                                start=(k == 0), stop=(k == KB1 - 1),
                            )
            for j in range(2):
                for m in range(KB2):
                    for s in range(2):
                        np_ = half * 4 + 2 * j + s
                        dst = pad_view(m, np_)[:, :, 1:15, 1:15]
                        src = (chain(grp[(j, m)], s)
                               .rearrange("p (i r c) -> p i r c",
                                          i=2, r=14, c=14))
                        if s == 0:
                            nc.vector.tensor_scalar(
                                dst, src, b1_t[:, m:m + 1], 0.0,
                                Alu.add, Alu.max)
                        else:
                            nc.scalar.activation(dst, src, Relu,
                                                 bias=b1_t[:, m:m + 1])

        # ---- conv2 (3x3, 256->256, pad 1) + bias + relu -> out2 ----------
        # Per np-quad: 8 chains in 4 pair tiles, contraction (tap, k) outer
        # with tap outermost so conv2 starts once the first w2 third lands.
        # Paired eviction: one strided op drains both chains of a tile.
        for half in range(2):
            grp = {}
            for j in range(2):
                for m in range(KB2):
                    grp[(j, m)] = psp.tile([P, 2 * SLOT], F32,
                                           name=f"ps2_{j}_{m}", tag="ps")
            for idx, (tap, k) in enumerate(
                    (tap, k) for tap in range(9) for k in range(KB2)):
                for j in range(2):
                    for m in range(KB2):
                        for s in range(2):
                            np_ = half * 4 + 2 * j + s
                            rhs = pad_view(k, np_)[:, :, tap // 3:tap // 3 + 14,
                                                   tap % 3:tap % 3 + 14]
                            nc.tensor.matmul(
                                chain(grp[(j, m)], s)
                                .rearrange("p (i r c) -> p i r c",
                                           i=2, r=14, c=14),
                                w2_t[tap][k][:, m * P:(m + 1) * P],
                                rhs,
                                start=(idx == 0), stop=(idx == 17),
                            )
            for j in range(2):
                for m in range(KB2):
                    np0 = half * 4 + 2 * j
                    dst = (out2[m][:, np0 * NF:(np0 + 2) * NF]
                           .rearrange("p (b c) -> p b c", b=2))
                    src = (grp[(j, m)][:]
                           .rearrange("p (b c) -> p b c", b=2)[:, :, 0:NF])
                    if j == 0:
                        nc.vector.tensor_scalar(
                            dst, src, b2_t[:, m:m + 1], 0.0, Alu.add, Alu.max)
                    else:
                        nc.scalar.activation(dst, src, Relu,
                                             bias=b2_t[:, m:m + 1])

        # ---- conv3 (1x1, 256->1024) + bias + residual + relu -> y --------
        # Per m: 8 chains in 4 pair tiles (pair j = images 4j..4j+3).
        # Pairs 0-2: DVE stt computes (psum+bias)+x into an SBUF pair, then
        # ReLU on ACT/GpSimd. Pair 3 keeps the residual on the PE as
        # identity-weight matmuls and evicts with a single strided relu op,
        # keeping PE/DVE/ACT/GpSimd balanced (~3us each per m-block).
        ID_J = 3
        for m in range(MB3):
            grp = pair_tiles(4, f"ps3_{m}")
            for k in range(KB2):
                for j in range(4):
                    for s in range(2):
                        np_ = 2 * j + s
                        stop = (k == KB2 - 1 and j != ID_J)
                        nc.tensor.matmul(
                            chain(grp[j], s),
                            w3_t[k][:, m * P:(m + 1) * P],
                            out2[k][:, np_ * NF:(np_ + 1) * NF],
                            start=(k == 0), stop=stop,
                        )
            for s in range(2):
                np_ = 2 * ID_J + s
                nc.tensor.matmul(
                    chain(grp[ID_J], s), id_t[:],
                    x_tiles[m][:, np_ * NF:(np_ + 1) * NF],
                    start=False, stop=True,
                )
            ystage = evp.tile([P, NLOC * HW], BF16, name="ystage",
                              tag="ystage", bufs=3)
            for j in range(4):
                np0 = 2 * j
                ydst = ystage[:, np0 * NF:(np0 + 2) * NF]
                if j == ID_J:
                    src = (grp[j][:]
                           .rearrange("p (b c) -> p b c", b=2)[:, :, 0:NF])
                    nc.scalar.activation(
                        ydst.rearrange("p (b c) -> p b c", b=2),
                        src, Relu, bias=b3_t[:, m:m + 1])
                else:
                    tsum = evp.tile([P, 2 * NF], F32, name="tsum",
                                    tag="tsum", bufs=6)
                    nc.vector.scalar_tensor_tensor(
                        tsum[:].rearrange("p (b c) -> p b c", b=2),
                        grp[j][:].rearrange("p (b c) -> p b c", b=2)[:, :, 0:NF],
                        b3_t[:, m:m + 1],
                        x_tiles[m][:, np0 * NF:(np0 + 2) * NF]
                        .rearrange("p (b c) -> p b c", b=2),
                        Alu.add, Alu.add)
                    if j == 1:
                        nc.gpsimd.tensor_scalar_max(ydst, tsum[:], 0.0)
                    else:
                        nc.scalar.activation(ydst, tsum[:], Relu, bias=0.0)
            nchunk = 4 if m == MB3 - 1 else 2
            CNF = NLOC * HW // nchunk
            for c in range(nchunk):
                nc.sync.dma_start(y_d.ap()[m][:, c * CNF:(c + 1) * CNF],
                                  ystage[:, c * CNF:(c + 1) * CNF])


def _prep(x, w1, g1, b1, m1, v1, w2, g2, b2, m2, v2, w3, g3, b3, m3, v3):
    """Host-side: fold BN, transpose weights to lhsT layouts, shard x."""
    def fold(w, g, b, m, v):
        scale = (g.astype(np.float64) / np.sqrt(v.astype(np.float64) + EPS))
        bias = b.astype(np.float64) - m.astype(np.float64) * scale
        wf = w.astype(np.float64) * scale.reshape(-1, *([1] * (w.ndim - 1)))
        return wf.astype(np.float32), bias.astype(np.float32)

    w1f, bias1 = fold(w1, g1, b1, m1, v1)   # [256,1024,1,1]
    w2f, bias2 = fold(w2, g2, b2, m2, v2)   # [256,256,3,3]
    w3f, bias3 = fold(w3, g3, b3, m3, v3)   # [1024,256,1,1]

    bf = ml_dtypes.bfloat16
    # lhsT SBUF images [P(=ci within kblock), ...]:
    # w1: [k, p, co] -> [p, (k co)]
    w1t = np.ascontiguousarray(
        w1f[:, :, 0, 0].T.reshape(KB1, P, WIDTH).transpose(1, 0, 2)
        .reshape(P, KB1 * WIDTH)).astype(bf)
    # w2: [tap, k, p, co] -> [p, (tap k co)], tap = dy*3+dx
    w2t = np.ascontiguousarray(
        w2f.transpose(2, 3, 1, 0).reshape(9 * KB2, P, WIDTH)
        .transpose(1, 0, 2).reshape(P, 9 * KB2 * WIDTH)).astype(bf)
    # w3: [k, p, co] -> [p, (k co)]
    w3t = np.ascontiguousarray(
        w3f[:, :, 0, 0].T.reshape(KB2, P, C_OUT).transpose(1, 0, 2)
        .reshape(P, KB2 * C_OUT)).astype(bf)

    b1h = bias1.reshape(KB2, P).T                          # [P, 2]
    b2h = bias2.reshape(KB2, P).T                          # [P, 2]
    b3h = bias3.reshape(MB3, P).T                          # [P, 8]
    ball = np.ascontiguousarray(
        np.concatenate([b1h, b2h, b3h], axis=1), dtype=np.float32)

    # x: [128, 1024, 14, 14] -> per core [2(half), KB1, P, NLOC*HW/2] bf16
    xs = (x.reshape(NCORES, NLOC, KB1, P, HW)
          .transpose(0, 2, 3, 1, 4)
          .reshape(NCORES, KB1, P, NLOC * HW)).astype(bf)
    H = NLOC * HW // 2
    xs = np.stack((xs[..., :H], xs[..., H:]), axis=1)  # [cores,2,KB1,P,H]

    common = {"w1t": w1t, "w2t": w2t, "w3t": w3t,
              "biases": ball, "ident": np.eye(P, dtype=np.float32).astype(bf)}
    in_maps = [dict(common, xt=np.ascontiguousarray(xs[i]))
               for i in range(NCORES)]
    return in_maps


def kernel(**inputs):
    inputs = {k: np.asarray(v) for k, v in inputs.items()}
    in_maps = _prep(**inputs)
    nc = _build()
    res = run_bass_kernel_spmd(nc, in_maps, core_ids=list(range(NCORES)))

    y = np.empty((NCORES * NLOC, C_OUT, 14, 14), dtype=np.float32)
    for i in range(NCORES):
        r = np.asarray(res.results[i]["y"], dtype=np.float32)  # [MB3,P,N*HW]
        r = (r.reshape(MB3, P, NLOC, HW)
             .transpose(2, 0, 1, 3)
             .reshape(NLOC, C_OUT, 14, 14))
        y[i * NLOC:(i + 1) * NLOC] = r
    return y


# revision 4
# speedup vs baseline: 1.5354x; 1.5354x over previous
"""Trainium2 Bass kernel for a ResNet Bottleneck block (inference).

Reference computation (NCHW, N=128, Cin=Cout=1024, width=256, H=W=14):
    out = relu(bn1(conv1x1(x, w1)))          # 1024 -> 256
    out = relu(bn2(conv3x3(out, w2, pad=1))) # 256 -> 256
    out = bn3(conv1x1(out, w3))              # 256 -> 1024
    y   = relu(out + x)

Strategy:
- Data-parallel: batch 128 sharded as 16 images per NeuronCore (8 cores),
  conv/BN params replicated. One NEFF, SPMD via run_bass_kernel_spmd.
- BN folded on host into per-channel weight scale + bias.
- All convs are matmuls on the TensorEngine with channels on the partition
  (contraction) dim. The 3x3 conv uses a zero-padded 16x16 per-image SBUF
  layout; each of the 9 taps is a shifted-window matmul accumulating in PSUM.
- Compute in bf16 (moving+stationary operands), fp32 PSUM accumulation,
  bf16 output (converted to fp32 on host).
- All input loads ride ONE HWDGE ring (sync) in exact consumption order:
  ring FIFO gives ordering for free at full HBM bandwidth, with no
  completion-chaining stalls. PE warm-up matmuls start right after the
  framework preamble so the HAM clock-gate lifts to 2.4 GHz before conv1.
- PSUM is managed as four [P, 1024] fp32 "pair" tiles (2 banks each): two
  accumulation chains per tile at column offsets 0 and 512. conv2/conv3
  evict two chains with ONE strided DVE/ACT op.
- Residual: 3 of 4 pairs per conv3 m-block go through DVE
  scalar_tensor_tensor (psum+bias)+x then ReLU on ACT/GpSimd; 1 pair stays
  on the PE as identity-weight matmuls so PE/DVE/ACT/GpSimd stay balanced.
"""

import sys

if "/opt/trn_rl_repo" not in sys.path:
    sys.path.insert(0, "/opt/trn_rl_repo")

import numpy as np
import ml_dtypes

import concourse.bass as bass
import concourse.bacc as bacc
import concourse.tile as tile
from concourse import mybir
from concourse.bass_utils import run_bass_kernel_spmd

EPS = 1e-5
NCORES = 8
NLOC = 16          # images per core
C_IN = 1024
WIDTH = 256
C_OUT = 1024
HW = 196           # 14*14
PADHW = 256        # 16*16 zero-padded image
P = 128
KB1 = C_IN // P    # 8 k-blocks for conv1 / residual channel blocks
KB2 = WIDTH // P   # 2 k-blocks for conv2/conv3 input
MB3 = C_OUT // P   # 8 m-blocks for conv3 output
NPAIRS = NLOC // 2  # 8 image pairs; N=392 per matmul
NF = 2 * HW        # 392
SLOT = 512         # fp32 columns per PSUM bank; chain s lives at s*SLOT

BF16 = mybir.dt.bfloat16
F32 = mybir.dt.float32
Relu = mybir.ActivationFunctionType.Relu

_cached = {}


def _build():
    """Build + compile the SPMD NEFF (one core's program). Cached."""
    if "nc" in _cached:
        return _cached["nc"]

    nc = bacc.Bacc("TRN2", target_bir_lowering=False, debug=False,
                   num_devices=NCORES)

    xt_d = nc.dram_tensor("xt", [2, KB1, P, NLOC * HW // 2], BF16,
                          kind="ExternalInput")
    # weights pre-arranged host-side as exact SBUF images (partition-major)
    w1_d = nc.dram_tensor("w1t", [P, KB1 * WIDTH], BF16, kind="ExternalInput")
    w2_d = nc.dram_tensor("w2t", [P, 9 * KB2 * WIDTH], BF16,
                          kind="ExternalInput")
    w3_d = nc.dram_tensor("w3t", [P, KB2 * C_OUT], BF16, kind="ExternalInput")
    b_d = nc.dram_tensor("biases", [P, 2 * KB2 + MB3], F32,
                         kind="ExternalInput")
    id_d = nc.dram_tensor("ident", [P, P], BF16, kind="ExternalInput")
    y_d = nc.dram_tensor("y", [MB3, P, NLOC * HW], BF16, kind="ExternalOutput")

    with tile.TileContext(nc) as tc:
        _emit(tc, nc, xt_d, w1_d, w2_d, w3_d, b_d, id_d, y_d)

    nc.compile()
    _cached["nc"] = nc
    return nc


def _emit(tc, nc, xt_d, w1_d, w2_d, w3_d, b_d, id_d, y_d):
    import contextlib
    from concourse.tile import add_dep_helper

    Alu = mybir.AluOpType

    with contextlib.ExitStack() as ctx:
        const = ctx.enter_context(tc.tile_pool(name="const", bufs=1))
        xpool = ctx.enter_context(tc.tile_pool(name="xpool", bufs=1))
        opool = ctx.enter_context(tc.tile_pool(name="opool", bufs=1))
        psp = ctx.enter_context(tc.tile_pool(name="psp", bufs=4, space="PSUM"))
        evp = ctx.enter_context(tc.tile_pool(name="evp", bufs=2))

        # ---- PE warm-up ---------------------------------------------------
        # The HAM clock gate needs ~3.4us of sustained PE activity to lift
        # the PE from 1.2 to 2.4 GHz, and conv1's first x tile only lands
        # ~5us after the preamble. Fill the gap with matmuls on a memset
        # scratch tile, alternating PSUM banks so they pipeline.
        scratch = const.tile([P, SLOT], BF16, name="scratch", tag="scratch")
        nc.gpsimd.memset(scratch[:], 0.0)
        warm_ps = psp.tile([P, 2 * SLOT], F32, name="warm_ps", tag="ps")
        for i in range(9):
            s = (i % 2) * SLOT
            nc.tensor.matmul(warm_ps[:, s:s + SLOT], scratch[:, 0:P],
                             scratch[:], start=True, stop=True)

        # ---- Input loads --------------------------------------------------
        # A single HWDGE ring processes transfers one at a time, and a lone
        # transfer ramps slowly (~150 GB/s for its first couple of us). So
        # the loads are interleaved across BOTH HWDGE rings (sync + scalar)
        # in consumption order: two transfers are always in flight, x k-
        # blocks land in order, and aggregate ingest saturates HBM early.
        # sync=False deps pin per-ring issue order without completion waits.
        ring_last = {}

        def ring(eng, dst, src):
            i = eng.dma_start(dst, src)
            if ring_last.get(eng.engine) is not None:
                add_dep_helper(i.ins, ring_last[eng.engine], sync=False,
                               reason="dma ring order")
            ring_last[eng.engine] = i.ins
            return i

        xsb = xpool.tile([P, KB1 * NLOC * HW], BF16, name="xsb", tag="xsb")
        x_tiles = [xsb[:, k * NLOC * HW:(k + 1) * NLOC * HW]
                   for k in range(KB1)]
        xv = xsb[:].rearrange("p (k h c) -> p k h c", k=KB1, h=2)

        w1sb = const.tile([P, KB1 * WIDTH], BF16, name="w1sb", tag="w1sb")
        w1_t = [w1sb[:, k * WIDTH:(k + 1) * WIDTH] for k in range(KB1)]
        w2sb = const.tile([P, 9 * KB2 * WIDTH], BF16, name="w2sb", tag="w2sb")
        w2_t = [[w2sb[:, (tap * KB2 + k) * WIDTH:(tap * KB2 + k + 1) * WIDTH]
                 for k in range(KB2)] for tap in range(9)]
        w3sb = const.tile([P, KB2 * C_OUT], BF16, name="w3sb", tag="w3sb")
        w3_t = [w3sb[:, k * C_OUT:(k + 1) * C_OUT] for k in range(KB2)]

        def xload(eng, half, k0, k1):
            ring(eng, xv[:, k0:k1, half, :],
                 xt_d.ap()[half][k0:k1].rearrange("k p c -> p k c"))

        W2C = 3 * KB2 * WIDTH
        # sync ring:            scalar ring:
        #   xA k0-1               w1 (whole)
        #   xA k4-5               xA k2-3
        #   xB k0-1               xA k6-7
        #   xB k4-5               xB k2-3
        #   w2 taps 0-2           xB k6-7
        #   w2 taps 6-8           w2 taps 3-5
        #   (y writes later)      w3
        xload(nc.sync, 0, 0, 2)
        ring(nc.scalar, w1sb[:], w1_d.ap())
        xload(nc.sync, 0, 4, 6)
        xload(nc.scalar, 0, 2, 4)
        xload(nc.sync, 1, 0, 2)
        xload(nc.scalar, 0, 6, 8)
        xload(nc.sync, 1, 4, 6)
        xload(nc.scalar, 1, 2, 4)
        ring(nc.sync, w2sb[:, 0:W2C], w2_d.ap()[:, 0:W2C])
        xload(nc.scalar, 1, 6, 8)
        ring(nc.sync, w2sb[:, 2 * W2C:], w2_d.ap()[:, 2 * W2C:])
        ring(nc.scalar, w2sb[:, W2C:2 * W2C], w2_d.ap()[:, W2C:2 * W2C])
        ring(nc.scalar, w3sb[:], w3_d.ap())

        # tiny constants go SWDGE (gpsimd) so they never block the rings
        ball = const.tile([P, 2 * KB2 + MB3], F32, name="ball", tag="ball")
        nc.gpsimd.dma_start(ball[:], b_d.ap())
        b1_t = ball[:, 0:KB2]
        b2_t = ball[:, KB2:2 * KB2]
        b3_t = ball[:, 2 * KB2:]
        id_t = const.tile([P, P], BF16, name="id_t", tag="id_t")
        nc.gpsimd.dma_start(id_t[:], id_d.ap())

        # Zero-padded conv1 output: per image a 16x16 field, payload at
        # rows/cols 1..14. Layout [P, NLOC*256].
        out1 = []
        for m in range(KB2):
            t = opool.tile([P, NLOC * PADHW], BF16, name=f"out1_{m}",
                           tag=f"out1_{m}")
            nc.vector.memset(t[:], 0.0)
            out1.append(t)

        out2 = []
        for m in range(KB2):
            t = opool.tile([P, NLOC * HW], BF16, name=f"out2_{m}",
                           tag=f"out2_{m}")
            out2.append(t)

        def pad_view(k, np_):
            return (out1[k][:, np_ * 2 * PADHW:(np_ + 1) * 2 * PADHW]
                    .rearrange("p (i r c) -> p i r c", i=2, r=16, c=16))

        def pair_tiles(n, tag):
            return [psp.tile([P, 2 * SLOT], F32, name=f"{tag}_{j}", tag="ps")
                    for j in range(n)]

        def chain(t, s):
            return t[:, s * SLOT:s * SLOT + NF]

        # ---- conv1 (1x1, 1024->256) + bias + relu -> padded out1 ---------
        # Per np-quad: 8 chains in 4 pair tiles (pairs j x m), k outer so
        # tiles fill as x k-blocks land. Evictions are per-chain (padded
        # 4D dst), split DVE/ACT so each tile drains in one op-latency.
        for half in range(2):
            grp = {}
            for j in range(2):
                for m in range(KB2):
                    grp[(j, m)] = psp.tile([P, 2 * SLOT], F32,
                                           name=f"ps1_{j}_{m}", tag="ps")
            for k in range(KB1):
                for j in range(2):
                    for m in range(KB2):
                        for s in range(2):
                            np_ = half * 4 + 2 * j + s
                            nc.tensor.matmul(
                                chain(grp[(j, m)], s),
                                w1_t[k][:, m * P:(m + 1) * P],
                                x_tiles[k][:, np_ * NF:(np_ + 1) * NF],
                                start=(k == 0), stop=(k == KB1 - 1),
                            )
            for j in range(2):
                for m in range(KB2):
                    for s in range(2):
                        np_ = half * 4 + 2 * j + s
                        dst = pad_view(m, np_)[:, :, 1:15, 1:15]
                        src = (chain(grp[(j, m)], s)
                               .rearrange("p (i r c) -> p i r c",
                                          i=2, r=14, c=14))
                        if s == 0:
                            nc.vector.tensor_scalar(
                                dst, src, b1_t[:, m:m + 1], 0.0,
                                Alu.add, Alu.max)
                        else:
                            nc.scalar.activation(dst, src, Relu,
                                                 bias=b1_t[:, m:m + 1])

        # ---- conv2 (3x3, 256->256, pad 1) + bias + relu -> out2 ----------
        # Per np-quad: 8 chains in 4 pair tiles, contraction (tap, k) outer
        # with tap outermost so conv2 starts once the first w2 third lands.
        # Paired eviction: one strided op drains both chains of a tile.
        for half in range(2):
            grp = {}
            for j in range(2):
                for m in range(KB2):
                    grp[(j, m)] = psp.tile([P, 2 * SLOT], F32,
                                           name=f"ps2_{j}_{m}", tag="ps")
            for idx, (tap, k) in enumerate(
                    (tap, k) for tap in range(9) for k in range(KB2)):
                for j in range(2):
                    for m in range(KB2):
                        for s in range(2):
                            np_ = half * 4 + 2 * j + s
                            rhs = pad_view(k, np_)[:, :, tap // 3:tap // 3 + 14,
                                                   tap % 3:tap % 3 + 14]
                            nc.tensor.matmul(
                                chain(grp[(j, m)], s)
                                .rearrange("p (i r c) -> p i r c",
                                           i=2, r=14, c=14),
                                w2_t[tap][k][:, m * P:(m + 1) * P],
                                rhs,
                                start=(idx == 0), stop=(idx == 17),
                            )
            for j in range(2):
                for m in range(KB2):
                    np0 = half * 4 + 2 * j
                    dst = (out2[m][:, np0 * NF:(np0 + 2) * NF]
                           .rearrange("p (b c) -> p b c", b=2))
                    src = (grp[(j, m)][:]
                           .rearrange("p (b c) -> p b c", b=2)[:, :, 0:NF])
                    if j == 0:
                        nc.vector.tensor_scalar(
                            dst, src, b2_t[:, m:m + 1], 0.0, Alu.add, Alu.max)
                    else:
                        nc.scalar.activation(dst, src, Relu,
                                             bias=b2_t[:, m:m + 1])

        # ---- conv3 (1x1, 256->1024) + bias + residual + relu -> y --------
        # Per m: 8 chains in 4 pair tiles (pair j = images 4j..4j+3).
        # Pairs 0-1: DVE stt computes (psum+bias)+x into an SBUF pair, then
        # ReLU on ACT. Pairs 2-3 keep the residual on the PE as identity-
        # weight matmuls and evict with one strided relu op each (DVE/ACT),
        # balancing PE (~3.3us) vs DVE (~3.0) vs ACT (~2.9) per m-block.
        # (GpSimd element-wise is an emulation path, ~11us/op — never used.)
        ID_JS = (2, 3)
        for m in range(MB3):
            grp = pair_tiles(4, f"ps3_{m}")
            for k in range(KB2):
                for j in range(4):
                    for s in range(2):
                        np_ = 2 * j + s
                        stop = (k == KB2 - 1 and j not in ID_JS)
                        nc.tensor.matmul(
                            chain(grp[j], s),
                            w3_t[k][:, m * P:(m + 1) * P],
                            out2[k][:, np_ * NF:(np_ + 1) * NF],
                            start=(k == 0), stop=stop,
                        )
            for j in ID_JS:
                for s in range(2):
                    np_ = 2 * j + s
                    nc.tensor.matmul(
                        chain(grp[j], s), id_t[:],
                        x_tiles[m][:, np_ * NF:(np_ + 1) * NF],
                        start=False, stop=True,
                    )
            ystage = evp.tile([P, NLOC * HW], BF16, name="ystage",
                              tag="ystage", bufs=3)
            for j in range(4):
                np0 = 2 * j
                ydst = ystage[:, np0 * NF:(np0 + 2) * NF]
                src = (grp[j][:]
                       .rearrange("p (b c) -> p b c", b=2)[:, :, 0:NF])
                if j in ID_JS:
                    if j == 2:
                        nc.vector.tensor_scalar(
                            ydst.rearrange("p (b c) -> p b c", b=2),
                            src, b3_t[:, m:m + 1], 0.0, Alu.add, Alu.max)
                    else:
                        nc.scalar.activation(
                            ydst.rearrange("p (b c) -> p b c", b=2),
                            src, Relu, bias=b3_t[:, m:m + 1])
                else:
                    tsum = evp.tile([P, 2 * NF], F32, name="tsum",
                                    tag="tsum", bufs=4)
                    nc.vector.scalar_tensor_tensor(
                        tsum[:].rearrange("p (b c) -> p b c", b=2),
                        src,
                        b3_t[:, m:m + 1],
                        x_tiles[m][:, np0 * NF:(np0 + 2) * NF]
                        .rearrange("p (b c) -> p b c", b=2),
                        Alu.add, Alu.add)
                    nc.scalar.activation(ydst, tsum[:], Relu, bias=0.0)
            nchunk = 4 if m == MB3 - 1 else 2
            CNF = NLOC * HW // nchunk
            for c in range(nchunk):
                nc.sync.dma_start(y_d.ap()[m][:, c * CNF:(c + 1) * CNF],
                                  ystage[:, c * CNF:(c + 1) * CNF])


def _prep(x, w1, g1, b1, m1, v1, w2, g2, b2, m2, v2, w3, g3, b3, m3, v3):
    """Host-side: fold BN, transpose weights to lhsT layouts, shard x."""
    def fold(w, g, b, m, v):
        scale = (g.astype(np.float64) / np.sqrt(v.astype(np.float64) + EPS))
        bias = b.astype(np.float64) - m.astype(np.float64) * scale
        wf = w.astype(np.float64) * scale.reshape(-1, *([1] * (w.ndim - 1)))
        return wf.astype(np.float32), bias.astype(np.float32)

    w1f, bias1 = fold(w1, g1, b1, m1, v1)   # [256,1024,1,1]
    w2f, bias2 = fold(w2, g2, b2, m2, v2)   # [256,256,3,3]
    w3f, bias3 = fold(w3, g3, b3, m3, v3)   # [1024,256,1,1]

    bf = ml_dtypes.bfloat16
    # lhsT SBUF images [P(=ci within kblock), ...]:
    # w1: [k, p, co] -> [p, (k co)]
    w1t = np.ascontiguousarray(
        w1f[:, :, 0, 0].T.reshape(KB1, P, WIDTH).transpose(1, 0, 2)
        .reshape(P, KB1 * WIDTH)).astype(bf)
    # w2: [tap, k, p, co] -> [p, (tap k co)], tap = dy*3+dx
    w2t = np.ascontiguousarray(
        w2f.transpose(2, 3, 1, 0).reshape(9 * KB2, P, WIDTH)
        .transpose(1, 0, 2).reshape(P, 9 * KB2 * WIDTH)).astype(bf)
    # w3: [k, p, co] -> [p, (k co)]
    w3t = np.ascontiguousarray(
        w3f[:, :, 0, 0].T.reshape(KB2, P, C_OUT).transpose(1, 0, 2)
        .reshape(P, KB2 * C_OUT)).astype(bf)

    b1h = bias1.reshape(KB2, P).T                          # [P, 2]
    b2h = bias2.reshape(KB2, P).T                          # [P, 2]
    b3h = bias3.reshape(MB3, P).T                          # [P, 8]
    ball = np.ascontiguousarray(
        np.concatenate([b1h, b2h, b3h], axis=1), dtype=np.float32)

    # x: [128, 1024, 14, 14] -> per core [2(half), KB1, P, NLOC*HW/2] bf16
    xs = (x.reshape(NCORES, NLOC, KB1, P, HW)
          .transpose(0, 2, 3, 1, 4)
          .reshape(NCORES, KB1, P, NLOC * HW)).astype(bf)
    H = NLOC * HW // 2
    xs = np.stack((xs[..., :H], xs[..., H:]), axis=1)  # [cores,2,KB1,P,H]

    common = {"w1t": w1t, "w2t": w2t, "w3t": w3t,
              "biases": ball, "ident": np.eye(P, dtype=np.float32).astype(bf)}
    in_maps = [dict(common, xt=np.ascontiguousarray(xs[i]))
               for i in range(NCORES)]
    return in_maps


def kernel(**inputs):
    inputs = {k: np.asarray(v) for k, v in inputs.items()}
    in_maps = _prep(**inputs)
    nc = _build()
    res = run_bass_kernel_spmd(nc, in_maps, core_ids=list(range(NCORES)))

    y = np.empty((NCORES * NLOC, C_OUT, 14, 14), dtype=np.float32)
    for i in range(NCORES):
        r = np.asarray(res.results[i]["y"], dtype=np.float32)  # [MB3,P,N*HW]
        r = (r.reshape(MB3, P, NLOC, HW)
             .transpose(2, 0, 1, 3)
             .reshape(NLOC, C_OUT, 14, 14))
        y[i * NLOC:(i + 1) * NLOC] = r
    return y
